# revision 1
# baseline (speedup 1.0000x reference)
"""ChebConv (K=3) GNN message passing on 8 TRN2 NeuronCores (Bass).

Math: with A = the 850k-edge multigraph adjacency (self-loops appended as
edges; row = dest, col = source), deg = in-degree by dest, dinv = deg^-1/2,
c[i] = #self-edges at node i (>= 1):
    spmm(v) = c * v - dinv * (A @ (dinv * v))
    T0 = x; T1 = spmm(x); T_{k+1} = 2*spmm(T_k) - T_{k-1}
    out = sum_k T_k @ W_k + bias

Distribution: nodes dest-sharded across 8 cores. Per hop each core gathers
pre-scaled source rows z = dinv*T (bf16, 512B rows) from a replicated DRAM
table via GPSIMD dma_gather (transpose mode: feature-on-partition,
edge-on-free), segment-sums per dest with DVE tensor_reduce over host-sorted
fixed-size-class segments, applies the Chebyshev recurrence, rebuilds the
next table shard via PE transpose + per-partition dinv scale on ACT, and
AllGathers it. Final phase computes out^T = sum W_k^T T_k on PE from bf16
feature-major copies; the host undoes the permutation/transposition.

dma_gather indices are int16, so two index passes address table rows
[0, 32768) and [NROWS-32768, NROWS); per-dest slot counts are equalized
across passes AND across cores (SPMD: one program for 8 cores) so all cores
share one static chunk/reduce structure; per-core variation lives in data.
"""

import numpy as np
import ml_dtypes

BF16 = ml_dtypes.bfloat16


class Cfg:
    def __init__(self, N=50000, E=800000, D=256, K=3, M=8, SC=1792,
                 DEST_CAP=320):
        self.N, self.E, self.D, self.K, self.M = N, E, D, K, M
        self.SC, self.DEST_CAP = SC, DEST_CAP
        self.NPC = N // M
        # table: per core [zero row, NPC permuted node rows]
        self.NROWS = M * (self.NPC + 1)
        self.RAB = min(32768, self.NROWS)
        self.B0 = self.NROWS - self.RAB
        self.PADB = self.RAB - self.NPC - 1  # last zero row, B-pass index
        self.NB = (self.NPC + 127) // 128


CFG = Cfg()


# ----------------------------------------------------------------------------
# host-side graph preprocessing
# ----------------------------------------------------------------------------

def build_plans(edge_index, cfg):
    N, M, NPC = cfg.N, cfg.M, cfg.NPC
    row = np.concatenate([edge_index[0], np.arange(N)]).astype(np.int64)
    col = np.concatenate([edge_index[1], np.arange(N)]).astype(np.int64)
    deg = np.bincount(row, minlength=N).astype(np.float64)
    dinv = deg ** -0.5
    cself = np.bincount(row[row == col], minlength=N).astype(np.float64)

    # pass eligibility at source-core granularity (table stride NPC+1)
    cA_max = -1
    while (cA_max + 2) * (NPC + 1) <= cfg.RAB:
        cA_max += 1
    cB_min = M
    while cB_min > 0 and (cB_min - 1) * (NPC + 1) >= cfg.B0:
        cB_min -= 1
    assert cB_min <= cA_max + 1, (cA_max, cB_min)

    src_core = col // NPC
    dest_core = row // NPC

    # ---- pass 1: per-core per-dest class sizes -----------------------------
    cores = []
    for m in range(M):
        sel = np.nonzero(dest_core == m)[0]
        r = row[sel] - m * NPC
        sc = src_core[sel]
        onlyA = sc < cB_min
        onlyB = sc > cA_max
        flex = ~onlyA & ~onlyB
        hA0 = np.bincount(r[onlyA], minlength=NPC)
        hB0 = np.bincount(r[onlyB], minlength=NPC)
        f = np.bincount(r[flex], minlength=NPC)
        fa = np.clip((hB0 - hA0 + f + 1) // 2, 0, f)
        s = np.maximum(np.maximum(hA0 + fa, hB0 + f - fa), 1)
        perm = np.argsort(s, kind="stable")
        cores.append(dict(sel=sel, r=r, col=col[sel], flex=flex, onlyB=onlyB,
                          fa=fa, perm=perm, s=s))

    # shared class sequence: per-rank max over cores
    s_shared = np.max(np.stack([c["s"][c["perm"]] for c in cores]), axis=0)
    s_shared = s_shared.astype(np.int64)

    # table row of each node under the per-core class-sort permutation
    grank = np.empty(N, dtype=np.int64)
    for m in range(M):
        rof = np.empty(NPC, dtype=np.int64)
        rof[cores[m]["perm"]] = np.arange(NPC)
        grank[m * NPC:(m + 1) * NPC] = m * (NPC + 1) + 1 + rof
    slot_base = np.zeros(NPC + 1, dtype=np.int64)
    np.cumsum(s_shared, out=slot_base[1:])
    total_slots = int(slot_base[-1])

    # shared chunking
    chunks = []
    d0 = 0
    while d0 < NPC:
        d1 = d0
        while (d1 < NPC and d1 - d0 < cfg.DEST_CAP
               and slot_base[d1 + 1] - slot_base[d0] <= cfg.SC):
            d1 += 1
        assert d1 > d0, "DEST_CAP/SC too small for a class"
        s0 = int(slot_base[d0])
        runs = []
        i = d0
        while i < d1:
            j = i
            while j < d1 and s_shared[j] == s_shared[i]:
                j += 1
            runs.append((int(s_shared[i]), j - i,
                         int(slot_base[i]) - s0, i - d0))
            i = j
        chunks.append(dict(d0=d0, nd=d1 - d0, s0=s0,
                           ns=int(slot_base[d1]) - s0, runs=runs))
        d0 = d1
    nch = len(chunks)

    # ---- pass 2: per-core slot filling -------------------------------------
    plans = []
    for m in range(M):
        c = cores[m]
        r, ccol, perm = c["r"], c["col"], c["perm"]
        rank_of = np.empty(NPC, dtype=np.int64)
        rank_of[perm] = np.arange(NPC)

        passB = np.zeros(len(r), dtype=bool)
        passB[c["onlyB"]] = True
        fi = np.nonzero(c["flex"])[0]
        fi = fi[np.argsort(r[fi], kind="stable")]
        rr = r[fi]
        starts = np.searchsorted(rr, np.arange(NPC))
        frank = np.arange(len(fi)) - starts[rr]
        passB[fi[frank >= c["fa"][rr]]] = True

        key = r * 2 + passB
        eorder = np.argsort(key, kind="stable")
        ke = key[eorder]
        kstarts = np.searchsorted(ke, np.arange(2 * NPC + 1))
        krank = np.arange(len(r)) - kstarts[ke]
        eslot = slot_base[rank_of[r[eorder]]] + krank
        grow = grank[ccol[eorder]]
        epB = passB[eorder]
        eidx = np.where(epB, grow - cfg.B0, grow)
        assert eidx.min() >= 0 and eidx.max() <= 32767, (
            eidx.min(), eidx.max())

        # int32 global-row offsets per slot (pad slots -> zero row 0)
        idxA = np.zeros(total_slots, dtype=np.int32)
        idxB = np.zeros(total_slots, dtype=np.int32)
        idxA[eslot[~epB]] = grow[~epB]
        idxB[eslot[epB]] = grow[epB]

        ng = cfg.SC // 128
        idxa_pack = np.zeros((nch, 128, ng), dtype=np.int32)
        idxb_pack = np.zeros((nch, 128, ng), dtype=np.int32)
        for ci, ch in enumerate(chunks):
            for pack, arr in ((idxa_pack, idxA), (idxb_pack, idxB)):
                buf = np.zeros(cfg.SC, dtype=np.int32)
                buf[:ch["ns"]] = arr[ch["s0"]:ch["s0"] + ch["ns"]]
                pack[ci] = buf.reshape(ng, 128).T

        plans.append(dict(perm=perm, idxa=idxa_pack, idxb=idxb_pack,
                          dinv_local=dinv[m * NPC:(m + 1) * NPC][perm],
                          c_local=cself[m * NPC:(m + 1) * NPC][perm]))

    g_perm = np.concatenate([m * NPC + plans[m]["perm"] for m in range(M)])
    return dict(plans=plans, dinv=dinv, g_perm=g_perm, chunks=chunks,
                nch=nch, total_slots=total_slots)


def build_inputs(x, weight, bias, prep, cfg):
    N, M, NPC, D, K = cfg.N, cfg.M, cfg.NPC, cfg.D, cfg.K
    dinv, g_perm = prep["dinv"], prep["g_perm"]

    zt0 = np.zeros((cfg.NROWS, D), dtype=BF16)
    zx = (x[g_perm] * dinv[g_perm][:, None]).astype(BF16)
    for m in range(M):
        base = m * (NPC + 1) + 1
        zt0[base:base + NPC] = zx[m * NPC:(m + 1) * NPC]

    wsb = np.zeros((128, (K + 1) * 4 * 128), dtype=BF16)
    for k in range(K + 1):
        for sub in range(2):
            for dh in range(2):
                base = ((k * 2 + sub) * 2 + dh) * 128
                wsb[:, base:base + 128] = weight[
                    k, sub * 128:(sub + 1) * 128,
                    dh * 128:(dh + 1) * 128].astype(BF16)
    bias_sb = np.stack([bias[:128], bias[128:]], axis=1).astype(np.float32)

    in_maps = []
    for m in range(M):
        p = prep["plans"][m]
        nodes = m * NPC + p["perm"]
        dl = p["dinv_local"].astype(np.float32)
        xs = x[nodes]
        xT = np.concatenate([xs[:, :128].T, xs[:, 128:].T],
                            axis=1).astype(np.float32)
        dinvrep = np.tile(np.concatenate([dl, dl])[None, :],
                          (128, 1)).astype(np.float32)
        cl = p["c_local"].astype(np.float32)
        crep = np.tile(np.concatenate([cl, cl])[None, :],
                       (128, 1)).astype(BF16)
        dcol = np.ones(cfg.NB * 128, dtype=np.float32)
        dcol[:NPC] = dl
        dinvcol = dcol.reshape(cfg.NB, 128).T.copy()
        in_maps.append(dict(
            zt0=zt0, xT=xT, xtbf=xT.astype(BF16), dinvrep=dinvrep,
            crep=crep, dinvcol=dinvcol, wsb=wsb, biassb=bias_sb,
            idxa=p["idxa"].reshape(-1, cfg.SC // 128),
            idxb=p["idxb"].reshape(-1, cfg.SC // 128),
        ))
    return in_maps


# ----------------------------------------------------------------------------
# device kernel builder
# ----------------------------------------------------------------------------

def build_nc(prep, cfg, finalize=True):
    import os
    from contextlib import ExitStack
    from concourse import bacc, mybir, tile, bass
    from concourse.masks import make_identity

    SKIP = os.environ.get("CHEB_SKIP", "")

    N, M, NPC, D, K = cfg.N, cfg.M, cfg.NPC, cfg.D, cfg.K
    SC, NB, NROWS, RAB = cfg.SC, cfg.NB, cfg.NROWS, cfg.RAB
    f32, bf16 = mybir.dt.float32, mybir.dt.bfloat16
    i32 = mybir.dt.int32
    Alu, Act, Ax = mybir.AluOpType, mybir.ActivationFunctionType, \
        mybir.AxisListType
    chunks, nch = prep["chunks"], prep["nch"]

    nc = bacc.Bacc("TRN2", target_bir_lowering=False, debug=False)

    din = {}
    for name, shape, dt in [
        ("zt0", [NROWS, D], bf16),
        ("xT", [128, 2 * NPC], f32),
        ("xtbf", [128, 2 * NPC], bf16),
        ("dinvrep", [128, 2 * NPC], f32),
        ("crep", [128, 2 * NPC], bf16),
        ("dinvcol", [128, NB], f32),
        ("wsb", [128, (K + 1) * 4 * 128], bf16),
        ("biassb", [128, 2], f32),
        ("idxa", [nch * 128, SC // 128], i32),
        ("idxb", [nch * 128, SC // 128], i32),
    ]:
        din[name] = nc.dram_tensor(name, shape, dt, kind="ExternalInput")
    outT = nc.dram_tensor("outT", [128, 2 * NPC], f32, kind="ExternalOutput")

    with tile.TileContext(nc) as tc, ExitStack() as ctx:
        dram = ctx.enter_context(tc.tile_pool(name="dram", bufs=1,
                                              space="DRAM"))
        zbufs = [dram.tile([NROWS, D], bf16, addr_space="Shared",
                           name=f"zbuf{h}") for h in (2, 3)]
        zshs = [dram.tile([NPC + 1, D], bf16, name=f"zsh{h}") for h in (1, 2)]
        t1f = dram.tile([128, 2 * NPC], f32, name="t1f")
        tkbf = [dram.tile([128, 2 * NPC], bf16, name=f"t{h}bf")
                for h in (1, 2, 3)]

        pers = ctx.enter_context(tc.tile_pool(name="pers", bufs=1))
        T_sb = pers.tile([128, 2 * NPC], f32)
        dinv_sb = pers.tile([128, 2 * NPC], f32)
        crep_sb = pers.tile([128, 2 * NPC], bf16)
        w_sb = pers.tile([128, (K + 1) * 4 * 128], bf16)
        bias_sb = pers.tile([128, 2], f32)
        dcol_sb = pers.tile([128, NB], f32)
        ident = pers.tile([128, 128], f32)
        zrow = pers.tile([1, D], bf16)

        nc.sync.dma_start(dinv_sb[:], din["dinvrep"][:])
        nc.sync.dma_start(crep_sb[:], din["crep"][:])
        nc.sync.dma_start(w_sb[:], din["wsb"][:])
        nc.sync.dma_start(bias_sb[:], din["biassb"][:])
        nc.sync.dma_start(dcol_sb[:], din["dinvcol"][:])
        make_identity(nc, ident[:])
        nc.gpsimd.memset(zrow[:], 0.0)
        for zs in zshs:
            nc.sync.dma_start(zs[0:1, :], zrow[:])

        io_pool = ctx.enter_context(tc.tile_pool(name="io", bufs=2))
        msg_pool = ctx.enter_context(tc.tile_pool(name="msg", bufs=2))
        red_pool = ctx.enter_context(tc.tile_pool(name="red", bufs=2))
        z_pool = ctx.enter_context(tc.tile_pool(name="zp", bufs=2))
        psum_pool = ctx.enter_context(
            tc.tile_pool(name="ps", bufs=4, space="PSUM"))

        dinv3 = dinv_sb[:].rearrange("p (u d) -> p u d", u=2)
        crep3 = crep_sb[:].rearrange("p (u d) -> p u d", u=2)
        T3 = T_sb[:].rearrange("p (u d) -> p u d", u=2)
        xT3 = din["xT"][:].rearrange("p (u d) -> p u d", u=2)
        t1f3 = t1f[:].rearrange("p (u d) -> p u d", u=2)

        NG = SC // 128
        for hop in (1, 2, 3):
            table = din["zt0"] if hop == 1 else zbufs[hop - 2]

            if "spmm" in SKIP:
                nc.vector.memset(T_sb[:], 0.0)
                for h in (1, 2, 3):
                    cb0 = io_pool.tile([128, 2, cfg.DEST_CAP], bf16, tag="cb")
                    nc.vector.memset(cb0[:], 0.0)
                break

            for ci, ch in enumerate(chunks):
                d0, nd, ns = ch["d0"], ch["nd"], ch["ns"]
                ia = io_pool.tile([128, NG], i32, tag="ia")
                ib = io_pool.tile([128, NG], i32, tag="ib")
                nc.sync.dma_start(ia[:], din["idxa"][ci * 128:(ci + 1) * 128])
                nc.sync.dma_start(ib[:], din["idxb"][ci * 128:(ci + 1) * 128])

                mt = msg_pool.tile([128, 4 * SC], bf16, tag="mt")
                mtv = mt[:].rearrange("p (q u s) -> p q u s", q=2, u=2)
                if "gather" in SKIP:
                    nc.vector.memset(mt[:], 0.0)
                else:
                    ngrp = (ch["ns"] + 127) // 128
                    for q, it in ((0, ia), (1, ib)):
                        for gi in range(ngrp):
                            gt = io_pool.tile([128, D], bf16, tag="gt",
                                              bufs=4)
                            nc.gpsimd.indirect_dma_start(
                                out=gt[:], out_offset=None, in_=table[:, :],
                                in_offset=bass.IndirectOffsetOnAxis(
                                    ap=it[:, gi:gi + 1], axis=0))
                            sl = slice(gi * 128, (gi + 1) * 128)
                            nc.sync.dma_start_transpose(
                                mtv[:, q, 0, sl], gt[:, 0:128])
                            nc.sync.dma_start_transpose(
                                mtv[:, q, 1, sl], gt[:, 128:256])

                ut = red_pool.tile([128, 2, cfg.DEST_CAP], f32, tag="ut")
                mt_ap = mt[:]
                for (s, n_c, soff, doff) in ch["runs"]:
                    # dims: [part][sub][dest][pass][elem]; reduce XY
                    rin = bass.AP(
                        mt_ap.tensor, mt_ap.offset + soff,
                        [mt_ap.ap[0], (SC, 2), (s, n_c), (2 * SC, 2),
                         (1, s)])
                    nc.vector.tensor_reduce(
                        out=ut[:, :, doff:doff + n_c], in_=rin,
                        axis=Ax.XY, op=Alu.add)

                tt = red_pool.tile([128, 2, cfg.DEST_CAP], f32, tag="tt")
                st = red_pool.tile([128, 2, cfg.DEST_CAP], f32, tag="st")
                nc.vector.tensor_mul(tt[:, :, :nd], ut[:, :, :nd],
                                     dinv3[:, :, d0:d0 + nd])
                if hop == 1:
                    xc = io_pool.tile([128, 2, cfg.DEST_CAP], f32, tag="xc")
                    nc.sync.dma_start(xc[:, :, :nd], xT3[:, :, d0:d0 + nd])
                    nc.vector.tensor_mul(st[:, :, :nd], crep3[:, :, d0:d0 + nd],
                                         xc[:, :, :nd])
                    nc.vector.tensor_sub(T3[:, :, d0:d0 + nd],
                                         st[:, :, :nd], tt[:, :, :nd])
                    nc.sync.dma_start(t1f3[:, :, d0:d0 + nd],
                                      T3[:, :, d0:d0 + nd])
                else:
                    nc.vector.tensor_mul(st[:, :, :nd], crep3[:, :, d0:d0 + nd],
                                         T3[:, :, d0:d0 + nd])
                    nc.vector.tensor_sub(st[:, :, :nd], st[:, :, :nd],
                                         tt[:, :, :nd])
                    pv = io_pool.tile([128, 2, cfg.DEST_CAP], f32, tag="xc")
                    src = xT3 if hop == 2 else t1f3
                    nc.sync.dma_start(pv[:, :, :nd], src[:, :, d0:d0 + nd])
                    nc.vector.scalar_tensor_tensor(
                        out=T3[:, :, d0:d0 + nd], in0=st[:, :, :nd],
                        scalar=2.0, in1=pv[:, :, :nd],
                        op0=Alu.mult, op1=Alu.subtract)
                cb = io_pool.tile([128, 2, cfg.DEST_CAP], bf16, tag="cb")
                nc.vector.tensor_copy(cb[:, :, :nd], T3[:, :, d0:d0 + nd])
                tb3 = tkbf[hop - 1][:].rearrange("p (u d) -> p u d", u=2)
                nc.sync.dma_start(tb3[:, :, d0:d0 + nd], cb[:, :, :nd])

            if hop < 3:
                # z-phase: transpose T to node-major, scale by dinv, AllGather
                for nb in range(NB):
                    w = min(128, NPC - nb * 128)
                    ps = psum_pool.tile([128, D], f32, tag="zt")
                    nc.tensor.transpose(
                        ps[:w, 0:128], T_sb[:, nb * 128:nb * 128 + w],
                        ident[:])
                    nc.tensor.transpose(
                        ps[:w, 128:256],
                        T_sb[:, NPC + nb * 128:NPC + nb * 128 + w], ident[:])
                    zt = z_pool.tile([128, D], bf16, tag="zs")
                    nc.scalar.activation(zt[:w], ps[:w], Act.Copy,
                                         scale=dcol_sb[:w, nb:nb + 1])
                    nc.sync.dma_start(
                        zshs[hop - 1][1 + nb * 128:1 + nb * 128 + w, :],
                        zt[:w])
                if "cc" in SKIP:
                    nc.sync.dma_start(zbufs[hop - 1][0:NPC + 1, :],
                                      zshs[hop - 1][:, :])
                else:
                    nc.gpsimd.collective_compute(
                        "AllGather", Alu.bypass,
                        replica_groups=[list(range(M))],
                        ins=[zshs[hop - 1][:, :].opt()],
                        outs=[zbufs[hop - 1][:, :].opt()],
                    )

        # final matmul: out^T[dout, tok] = sum_k W_k^T T_k + bias
        TOKC = 384
        for t0 in range(0, NPC, TOKC):
            w = min(TOKC, NPC - t0)
            tks = []
            for k in range(K + 1):
                tk = io_pool.tile([128, 2, TOKC], bf16, tag=f"mk{k}")
                src = din["xtbf"][:] if k == 0 else tkbf[k - 1][:]
                s3 = src.rearrange("p (u d) -> p u d", u=2)
                nc.sync.dma_start(tk[:, :, :w], s3[:, :, t0:t0 + w])
                tks.append(tk)
            for dh in range(2):
                pm = psum_pool.tile([128, TOKC], f32, tag="pm")
                first = True
                for k in range(K + 1):
                    for sub in range(2):
                        base = ((k * 2 + sub) * 2 + dh) * 128
                        nc.tensor.matmul(
                            pm[:, :w], lhsT=w_sb[:, base:base + 128],
                            rhs=tks[k][:, sub, :w],
                            start=first, stop=(k == K and sub == 1))
                        first = False
                stage = z_pool.tile([128, TOKC], f32, tag="og")
                nc.scalar.activation(stage[:, :w], pm[:, :w], Act.Identity,
                                     bias=bias_sb[:, dh:dh + 1])
                nc.sync.dma_start(
                    outT[:, dh * NPC + t0: dh * NPC + t0 + w], stage[:, :w])

    if finalize:
        nc.finalize()
    return nc


# ----------------------------------------------------------------------------
# entry point
# ----------------------------------------------------------------------------

def _run(x, edge_index, weight, bias, cfg, use_sim=False):
    from concourse.bass_utils import run_bass_kernel_spmd

    x = np.asarray(x, dtype=np.float32)
    edge_index = np.asarray(edge_index)
    weight = np.asarray(weight, dtype=np.float32)
    bias = np.asarray(bias, dtype=np.float32)

    prep = build_plans(edge_index, cfg)
    in_maps = build_inputs(x, weight, bias, prep, cfg)
    nc = build_nc(prep, cfg, finalize=True)

    if use_sim:
        from concourse import bass_interp
        sim = bass_interp.MultiCoreSim(nc, cfg.M)
        for m in range(cfg.M):
            for k, v in in_maps[m].items():
                sim.cores[m].tensor(k)[:] = v
        sim.simulate()
        results = [{"outT": sim.cores[m].mem_tensor("outT")}
                   for m in range(cfg.M)]
    else:
        res = run_bass_kernel_spmd(nc, in_maps, list(range(cfg.M)))
        results = res.results

    NPC = cfg.NPC
    out = np.empty((cfg.N, cfg.D), dtype=np.float32)
    for m in range(cfg.M):
        oT = np.asarray(results[m]["outT"], dtype=np.float32)
        # oT[p, dh, i] = out[node i][dh*128 + p]  ->  [NPC, 256]
        o = oT.reshape(128, 2, NPC).transpose(2, 1, 0).reshape(NPC, cfg.D)
        out[m * NPC + prep["plans"][m]["perm"]] = o
    return out


def kernel(x, edge_index, weight, bias):
    return _run(x, edge_index, weight, bias, CFG, use_sim=False)

